# revision 7
# baseline (speedup 1.0000x reference)
"""Trainium2 Bass kernel for per-sample channel-modulated 3x3 conv (CoModConv).

Math (matches the reference nn.Module):
    s = lrelu(lrelu(lrelu(y @ w0.T + b0) @ w1.T + b1) @ w2.T + b2)   # (B, C_in)
    out = conv3x3(x * s[:, :, None, None], conv_w, pad=1)            # (B, C_out, H, W)

Strategy: data-parallel over batch, 2 samples per NeuronCore (8 cores), with a
1-D Winograd F(4,3) transform along H (direct 3-tap conv along W), all in fp16:
  - Host precomputes U0 = G @ conv_w (transform of the shared weight along kh);
    the per-sample channel scale s folds into U on device (one tensor_scalar
    per (sample, ci-tile)), so modulation is free.
  - Device builds V = B^T d (6 transformed row-planes per input tile of 4 rows)
    on the vector engine (fp16, 2x mode), with the constant-scale steps on the
    scalar engine.
  - The conv becomes, per (sample, co_t, 8-i-tile chunk), 36 accumulating
    128x128x512 fp16 matmuls (ci_t x kw x u) into 6 PSUM banks -- 288 matmuls
    per core vs 576 for direct conv (1.5x fewer after transform overhead;
    2.25x fewer MACs land in 6/9 of the direct kernel taps).
  - Inverse transform A^T m (6 -> 4 output rows) runs on scalar-engine drains
    (PSUM fp32 -> fp16) + 13 small vector ops per chunk, writing interleaved
    output rows; outputs DMA out in fp16 and are upcast on host.
fp16 (not bf16) keeps the Winograd transform numerics comfortably inside the
tolerance: measured rel err ~4.6e-3 vs ~4.3e-2 for bf16.
"""

import numpy as np

B, D_CAT, C_IN, C_OUT, K, H, W = 16, 512, 256, 256, 3, 64, 64
NCORES = 8
BL = B // NCORES          # samples per core (2)
CIT = C_IN // 128         # ci tiles (2)
COT = C_OUT // 128        # co tiles (2)
GH = H + 2                # padded grid rows (66)
GW = W + 2                # padded grid cols (66)
UD = 6                    # F(4,3) transform length
IT = 16                   # winograd i-tiles along H (4 output rows each)
CHI = 8                   # i-tiles per PSUM chunk (8*64 = 512 cols)
NCH = IT // CHI           # chunks per (sample, co_t) (2)
UBF = UD * K * 128        # Ub columns per co_t block (2304)

# packed MLP-param column offsets (per partition). Weights and y ship in bf16
# (pp1 = y + w0, pp2 = w1, pp3 = w2, ordered by first use); biases in fp32.
_PY = 0                       # y^T:   4 k-tiles x BL
_PW0 = _PY + 4 * BL           # w0^T:  4 k-tiles x 256
_P1TOT = _PW0 + 4 * C_IN
_P2TOT = 2 * C_IN             # w1^T
_P3TOT = 2 * C_IN             # w2^T
_NBIAS = 3 * CIT              # b0, b1, b2 per ci-tile (fp32)

# F(4,3) weight transform (G), with the sign of row u=1 folded in because the
# device computes V[1] = 4(d1+d2) - (d3+d4) = -B^T[1] d.
_G = np.array(
    [
        [1 / 4, 0, 0],
        [-1 / 6, -1 / 6, -1 / 6],
        [-1 / 6, 1 / 6, -1 / 6],
        [1 / 24, 1 / 12, 1 / 6],
        [1 / 24, -1 / 12, 1 / 6],
        [0, 0, 1],
    ],
    dtype=np.float64,
) * np.array([1, -1, 1, 1, 1, 1], dtype=np.float64)[:, None]

_COMPILED = None


def _build():
    import concourse.mybir as mybir
    import concourse.tile as tile
    from concourse import bacc

    bf16 = mybir.dt.bfloat16
    f16 = mybir.dt.float16
    f32 = mybir.dt.float32
    Prelu = mybir.ActivationFunctionType.Prelu
    ADD = mybir.AluOpType.add
    SUB = mybir.AluOpType.subtract

    nc = bacc.Bacc("TRN2", target_bir_lowering=False, debug=False, num_devices=NCORES)

    pp1_in = nc.declare_dram_parameter("pp1", [128, _P1TOT], bf16, isOutput=False)
    pp2_in = nc.declare_dram_parameter("pp2", [128, _P2TOT], bf16, isOutput=False)
    pp3_in = nc.declare_dram_parameter("pp3", [128, _P3TOT], bf16, isOutput=False)
    bias_in = nc.declare_dram_parameter("bias", [128, _NBIAS], f32, isOutput=False)
    u0_in = nc.declare_dram_parameter("u0", [CIT, 128, COT * UBF], f16, isOutput=False)
    xg_in = nc.declare_dram_parameter("xg", [BL, CIT, 128, GH * GW], f16, isOutput=False)
    out_ext = nc.declare_dram_parameter("out", [BL, COT, 128, H * W], f16, isOutput=True)

    with tile.TileContext(nc) as tc:
        with (
            tc.tile_pool(name="const", bufs=1) as cpool,
            tc.tile_pool(name="grid", bufs=2) as gpool,
            tc.tile_pool(name="vpool", bufs=4) as vpool,
            tc.tile_pool(name="tmp", bufs=2) as tpool,
            tc.tile_pool(name="minv", bufs=2) as mpool,
            tc.tile_pool(name="oout", bufs=2) as opool,
            tc.tile_pool(name="cpsum", bufs=8, space="PSUM") as cpsum,
        ):
            # warm the scalar-engine activation table before the params land
            warm = cpool.tile([128, 1], f32)
            nc.vector.memset(warm[:], 0.0)
            nc.scalar.activation(warm[:], warm[:], Prelu, bias=warm[:], scale=1.0, alpha=0.01)

            # ---- DMAs: MLP params on the HWDGE path; x grids and U0 via the
            # gpsimd SWDGE queue so they don't serialize behind the params ----
            pp1_sb = cpool.tile([128, _P1TOT], bf16)
            nc.sync.dma_start(pp1_sb[:], pp1_in[:])
            bias_sb = cpool.tile([128, _NBIAS], f32)
            nc.sync.dma_start(bias_sb[:], bias_in[:])
            pp2_sb = cpool.tile([128, _P2TOT], bf16)
            nc.sync.dma_start(pp2_sb[:], pp2_in[:])
            pp3_sb = cpool.tile([128, _P3TOT], bf16)
            nc.sync.dma_start(pp3_sb[:], pp3_in[:])

            grids = {}
            u0_sbs = {}

            def load_grid(b, ci_t):
                t = gpool.tile([128, GH * GW], f16, tag="g")
                nc.gpsimd.dma_start(t[:], xg_in[b, ci_t])
                grids[(b, ci_t)] = t[:].rearrange("p (h w) -> p h w", w=GW)

            load_grid(0, 0)
            for ci_t in range(CIT):
                t = cpool.tile([128, COT * UBF], f16, tag=f"u0{ci_t}")
                nc.gpsimd.dma_start(t[:], u0_in[ci_t])
                u0_sbs[ci_t] = t
            load_grid(0, 1)

            # ---- style MLP (identical to the direct-conv kernel) ----
            def mlp_layer(rhs_of_kt, kts, w_sb, w_base, bias_ap, out_sb):
                for ct in range(CIT):
                    mps = cpsum.tile([128, 512], f32, tag="ps")
                    for kt in range(kts):
                        nc.tensor.matmul(
                            mps[:, :BL],
                            w_sb[:, w_base + kt * C_IN + ct * 128 :][:, :128],
                            rhs_of_kt(kt),
                            start=(kt == 0),
                            stop=(kt == kts - 1),
                        )
                    nc.scalar.activation(
                        out_sb[:, ct * BL : (ct + 1) * BL],
                        mps[:, :BL],
                        Prelu,
                        bias=bias_ap(ct),
                        scale=1.0,
                        alpha=0.01,
                    )

            s0_sb = cpool.tile([128, CIT * BL], bf16)
            s1_sb = cpool.tile([128, CIT * BL], bf16)
            s_sb = cpool.tile([128, CIT * BL], f32)
            mlp_layer(
                lambda kt: pp1_sb[:, _PY + kt * BL : _PY + (kt + 1) * BL],
                4, pp1_sb, _PW0,
                lambda ct: bias_sb[:, ct : ct + 1],
                s0_sb,
            )
            mlp_layer(
                lambda kt: s0_sb[:, kt * BL : (kt + 1) * BL],
                2, pp2_sb, 0,
                lambda ct: bias_sb[:, CIT + ct : CIT + ct + 1],
                s1_sb,
            )
            mlp_layer(
                lambda kt: s1_sb[:, kt * BL : (kt + 1) * BL],
                2, pp3_sb, 0,
                lambda ct: bias_sb[:, 2 * CIT + ct : 2 * CIT + ct + 1],
                s_sb,
            )

            # ---- modulated transformed weights: Ub = U0 * s[b, ci] (fp16) ----
            ub = {}
            for b in range(BL):
                for ci_t in range(CIT):
                    t = cpool.tile([128, COT * UBF], f16, tag=f"ub{b}{ci_t}")
                    nc.vector.tensor_scalar_mul(
                        t[:], u0_sbs[ci_t][:],
                        s_sb[:, ci_t * BL + b : ci_t * BL + b + 1],
                    )
                    ub[(b, ci_t)] = t

            # ---- V = B^T d row-transform per (sample, ci-tile) ----
            vs = {}

            def build_v(b, ci_t):
                g = grids[(b, ci_t)]

                def d(p):          # rows p, p+4, ..., p+60  -> [128, 16, 66]
                    return g[:, p : p + 61 : 4, :]

                v = vpool.tile([128, UD * IT * GW], f16, tag="v")
                vv = v[:].rearrange("p (u i w) -> p u i w", i=IT, w=GW)

                def tmp(tag):
                    t = tpool.tile([128, IT * GW], f16, tag=tag)
                    return t[:].rearrange("p (i w) -> p i w", w=GW)

                t1, t2, t3, t4 = tmp("t1"), tmp("t2"), tmp("t3"), tmp("t4")
                t6, t7, t8 = tmp("t6"), tmp("t7"), tmp("t8")
                q1, q2, q3 = tmp("q1"), tmp("q2"), tmp("q3")
                tt = nc.vector.tensor_tensor
                tt(t1, d(1), d(2), ADD)
                tt(t3, d(1), d(2), SUB)
                nc.scalar.mul(q1, t1, 4.0)
                tt(t2, d(3), d(4), ADD)
                tt(vv[:, 1], q1, t2, SUB)          # V1 = 4(d1+d2)-(d3+d4) = -w1
                nc.scalar.mul(q2, t3, 4.0)
                tt(t4, d(3), d(4), SUB)
                tt(vv[:, 2], q2, t4, SUB)          # V2 = 4(d1-d2)-(d3-d4)
                tt(t6, d(3), d(1), SUB)
                nc.scalar.mul(q3, t6, 2.0)
                tt(t7, d(4), d(2), SUB)
                tt(vv[:, 3], q3, t7, ADD)          # V3 = 2(d3-d1)+(d4-d2)
                tt(vv[:, 4], t7, q3, SUB)          # V4 = (d4-d2)-2(d3-d1)
                nc.scalar.mul(q1, d(0), 4.0)
                nc.scalar.mul(q2, d(2), 5.0)
                tt(t8, q1, q2, SUB)
                tt(vv[:, 0], t8, d(4), ADD)        # V0 = 4d0-5d2+d4
                nc.scalar.mul(q1, d(1), 4.0)
                nc.scalar.mul(q2, d(3), 5.0)
                tt(t8, q1, q2, SUB)
                tt(vv[:, 5], t8, d(5), ADD)        # V5 = 4d1-5d3+d5
                vs[(b, ci_t)] = vv

            build_v(0, 0)
            build_v(0, 1)

            # ---- conv + inverse transform per (sample, co_t, chunk) ----
            def conv_chunk(b, co_t, ch):
                ps = [
                    cpsum.tile([128, CHI * W], f32, name=f"ps_{b}_{co_t}_{ch}_{u}", tag="ps")
                    for u in range(UD)
                ]
                for ci_t in range(CIT):
                    vv = vs[(b, ci_t)]
                    w_sb = ub[(b, ci_t)]
                    for kw in range(K):
                        for u in range(UD):
                            nc.tensor.matmul(
                                ps[u][:],
                                w_sb[:, ((co_t * UD + u) * K + kw) * 128 :][:, :128],
                                vv[:, u, CHI * ch : CHI * ch + CHI, kw : kw + W],
                                start=(ci_t == 0 and kw == 0),
                                stop=(ci_t == CIT - 1 and kw == K - 1),
                            )
                # drains: PSUM fp32 -> fp16 on the scalar engine
                m = []
                for u in range(UD):
                    t = mpool.tile(
                        [128, CHI * W], f16, name=f"m_{b}_{co_t}_{ch}_{u}", tag=f"m{u}"
                    )
                    nc.gpsimd.tensor_copy(t[:], ps[u][:])
                    m.append(t[:])
                # inverse transform A^T m on DVE (fp16, 2x/4x modes)
                def itmp(tag):
                    t = mpool.tile(
                        [128, CHI * W], f16, name=f"{tag}_{b}_{co_t}_{ch}", tag=tag
                    )
                    return t[:]

                p_, q_, r_, s_ = itmp("ip"), itmp("iq"), itmp("ir"), itmp("is")
                s2, r4, s8 = itmp("s2"), itmp("r4"), itmp("s8")
                oa, ob = itmp("oa"), itmp("ob")
                tt = nc.vector.tensor_tensor
                # U row 1 and device V1 are both sign-flipped, so m[] here are
                # the true Winograd products and A^T applies directly.
                tt(p_, m[1], m[2], ADD)            # p = m1 + m2
                tt(q_, m[1], m[2], SUB)            # q = m1 - m2
                tt(r_, m[3], m[4], ADD)
                tt(s_, m[3], m[4], SUB)
                nc.scalar.mul(s2, s_, 2.0)
                nc.scalar.mul(r4, r_, 4.0)
                nc.scalar.mul(s8, s_, 8.0)
                o = opool.tile([128, 4 * CHI * W], f16, tag="o")
                ov = o[:].rearrange("p (h w) -> p h w", w=W)
                nr = 4 * CHI                        # 32 output rows per chunk
                tt(oa, m[0], p_, ADD)
                tt(ov[:, 0 : nr - 3 : 4, :], oa, r_, ADD)       # o0 = m0+p+r
                tt(ov[:, 1 : nr - 2 : 4, :], q_, s2, ADD)       # o1 = q+2s
                tt(ov[:, 2 : nr - 1 : 4, :], p_, r4, ADD)       # o2 = p+4r
                tt(ob, q_, m[5], ADD)
                tt(ov[:, 3 : nr : 4, :], ob, s8, ADD)           # o3 = q+8s+m5
                nc.sync.dma_start(
                    out_ext[b, co_t][:, 2048 * ch : 2048 * (ch + 1)], o[:]
                )

            for co_t in range(COT):
                if co_t == 1:
                    load_grid(1, 0)
                    build_v(1, 0)
                    load_grid(1, 1)
                    build_v(1, 1)
                for ch in range(NCH):
                    conv_chunk(0, co_t, ch)
            for co_t in range(COT):
                for ch in range(NCH):
                    conv_chunk(1, co_t, ch)

    nc.compile()
    return nc


def _get_nc():
    global _COMPILED
    if _COMPILED is None:
        _COMPILED = _build()
    return _COMPILED


def _prep_in_maps(x, y, w0, b0, w1, b1, w2, b2, conv_w):
    import ml_dtypes

    BF = ml_dtypes.bfloat16
    x = np.ascontiguousarray(x, dtype=np.float32)
    y = np.ascontiguousarray(y, dtype=np.float32)

    # packed per-core-invariant MLP params (bf16 weights, fp32 biases)
    pp1_shared = np.empty((128, _P1TOT), dtype=BF)
    pp1_shared[:, _PW0 : _PW0 + 4 * C_IN] = (
        w0.astype(np.float32).T.reshape(4, 128, C_IN).transpose(1, 0, 2).reshape(128, 4 * C_IN)
    ).astype(BF)
    pp2 = np.ascontiguousarray(
        w1.astype(np.float32).T.reshape(2, 128, C_IN).transpose(1, 0, 2).reshape(128, 2 * C_IN)
    ).astype(BF)
    pp3 = np.ascontiguousarray(
        w2.astype(np.float32).T.reshape(2, 128, C_IN).transpose(1, 0, 2).reshape(128, 2 * C_IN)
    ).astype(BF)
    bias = np.empty((128, _NBIAS), dtype=np.float32)
    for i, bb in enumerate((b0, b1, b2)):
        bias[:, i * CIT : (i + 1) * CIT] = bb.astype(np.float32).reshape(CIT, 128).T

    # U0 = G @ conv_w along kh: (O,I,kh,kw) -> (I_t, ci, co_t, u, kw, co)
    T = np.einsum("uh,oihw->oiuw", _G, conv_w.astype(np.float64))
    u0 = np.ascontiguousarray(
        T.transpose(1, 2, 3, 0)
        .reshape(CIT, 128, UD, K, COT, 128)
        .transpose(0, 1, 4, 2, 3, 5)
        .reshape(CIT, 128, COT * UBF)
    ).astype(np.float16)

    xg_all = np.zeros((B, CIT, 128, GH, GW), dtype=np.float16)
    xg_all[:, :, :, 1 : H + 1, 1 : W + 1] = x.reshape(B, CIT, 128, H, W).astype(np.float16)
    xg_all = xg_all.reshape(B, CIT, 128, GH * GW)

    in_maps = []
    for c in range(NCORES):
        sl = slice(c * BL, (c + 1) * BL)
        pp1 = pp1_shared.copy()
        pp1[:, _PY : _PY + 4 * BL] = (
            y[sl].T.reshape(4, 128, BL).transpose(1, 0, 2).reshape(128, 4 * BL)
        ).astype(BF)
        in_maps.append(
            {
                "pp1": pp1,
                "pp2": pp2,
                "pp3": pp3,
                "bias": bias,
                "u0": u0,
                "xg": np.ascontiguousarray(xg_all[sl]),
            }
        )
    return in_maps


def _run(in_maps, trace=False):
    from concourse.bass_utils import run_bass_kernel_spmd

    nc = _get_nc()
    res = run_bass_kernel_spmd(nc, in_maps, list(range(NCORES)), trace=trace)
    out = np.concatenate(
        [
            res.results[c]["out"].astype(np.float32).reshape(BL, C_OUT, H, W)
            for c in range(NCORES)
        ],
        axis=0,
    )
    return out, res


def kernel(x, y, w0, b0, w1, b1, w2, b2, conv_w):
    in_maps = _prep_in_maps(x, y, w0, b0, w1, b1, w2, b2, conv_w)
    out, _ = _run(in_maps, trace=False)
    return out


# revision 8
# speedup vs baseline: 1.1698x; 1.1698x over previous
"""Trainium2 Bass kernel for per-sample channel-modulated 3x3 conv (CoModConv).

Math (matches the reference nn.Module):
    s = lrelu(lrelu(lrelu(y @ w0.T + b0) @ w1.T + b1) @ w2.T + b2)   # (B, C_in)
    out = conv3x3(x * s[:, :, None, None], conv_w, pad=1)            # (B, C_out, H, W)

Strategy: data-parallel over batch, 2 samples per NeuronCore (8 cores), with a
1-D Winograd F(4,3) transform along H (direct 3-tap conv along W), all in fp16:
  - Host precomputes U0 = G @ conv_w (transform of the shared weight along kh);
    the per-sample channel scale s folds into U on device (one tensor_scalar
    per (sample, ci-tile)), so modulation is free.
  - Device builds V = B^T d (6 transformed row-planes per input tile of 4 rows)
    on the vector engine (fp16, 2x mode), with the constant-scale steps on the
    scalar engine.
  - The conv becomes, per (sample, co_t, 8-i-tile chunk), 36 accumulating
    128x128x512 fp16 matmuls (ci_t x kw x u) into 6 PSUM banks -- 288 matmuls
    per core vs 576 for direct conv (1.5x fewer after transform overhead;
    2.25x fewer MACs land in 6/9 of the direct kernel taps).
  - Inverse transform A^T m (6 -> 4 output rows) runs on scalar-engine drains
    (PSUM fp32 -> fp16) + 13 small vector ops per chunk, writing interleaved
    output rows; outputs DMA out in fp16 and are upcast on host.
fp16 (not bf16) keeps the Winograd transform numerics comfortably inside the
tolerance: measured rel err ~4.6e-3 vs ~4.3e-2 for bf16.
"""

import numpy as np

B, D_CAT, C_IN, C_OUT, K, H, W = 16, 512, 256, 256, 3, 64, 64
NCORES = 8
BL = B // NCORES          # samples per core (2)
CIT = C_IN // 128         # ci tiles (2)
COT = C_OUT // 128        # co tiles (2)
GH = H + 2                # padded grid rows (66)
GW = W + 2                # padded grid cols (66)
UD = 6                    # F(4,3) transform length
IT = 16                   # winograd i-tiles along H (4 output rows each)
CHI = 8                   # i-tiles per PSUM chunk (8*64 = 512 cols)
NCH = IT // CHI           # chunks per (sample, co_t) (2)
UBF = UD * K * 128        # Ub columns per co_t block (2304)

# packed MLP-param column offsets (per partition). Weights and y ship in bf16
# (pp1 = y + w0, pp2 = w1, pp3 = w2, ordered by first use); biases in fp32.
_PY = 0                       # y^T:   4 k-tiles x BL
_PW0 = _PY + 4 * BL           # w0^T:  4 k-tiles x 256
_P1TOT = _PW0 + 4 * C_IN
_P2TOT = 2 * C_IN             # w1^T
_P3TOT = 2 * C_IN             # w2^T
_NBIAS = 3 * CIT              # b0, b1, b2 per ci-tile (fp32)

# F(4,3) weight transform (G), with the sign of row u=1 folded in because the
# device computes V[1] = 4(d1+d2) - (d3+d4) = -B^T[1] d.
_G = np.array(
    [
        [1 / 4, 0, 0],
        [-1 / 6, -1 / 6, -1 / 6],
        [-1 / 6, 1 / 6, -1 / 6],
        [1 / 24, 1 / 12, 1 / 6],
        [1 / 24, -1 / 12, 1 / 6],
        [0, 0, 1],
    ],
    dtype=np.float64,
) * np.array([1, -1, 1, 1, 1, 1], dtype=np.float64)[:, None]

_COMPILED = None


def _build():
    import concourse.mybir as mybir
    import concourse.tile as tile
    from concourse import bacc

    bf16 = mybir.dt.bfloat16
    f16 = mybir.dt.float16
    f32 = mybir.dt.float32
    Prelu = mybir.ActivationFunctionType.Prelu
    ADD = mybir.AluOpType.add
    SUB = mybir.AluOpType.subtract

    nc = bacc.Bacc("TRN2", target_bir_lowering=False, debug=False, num_devices=NCORES)

    pp1_in = nc.declare_dram_parameter("pp1", [128, _P1TOT], bf16, isOutput=False)
    pp2_in = nc.declare_dram_parameter("pp2", [128, _P2TOT], bf16, isOutput=False)
    pp3_in = nc.declare_dram_parameter("pp3", [128, _P3TOT], bf16, isOutput=False)
    bias_in = nc.declare_dram_parameter("bias", [128, _NBIAS], f32, isOutput=False)
    u0_in = nc.declare_dram_parameter("u0", [CIT, 128, COT * UBF], f16, isOutput=False)
    xg_in = nc.declare_dram_parameter("xg", [BL, CIT, 128, GH * GW], f16, isOutput=False)
    out_ext = nc.declare_dram_parameter("out", [BL, COT, 128, H * W], f16, isOutput=True)

    with tile.TileContext(nc) as tc:
        with (
            tc.tile_pool(name="const", bufs=1) as cpool,
            tc.tile_pool(name="grid", bufs=2) as gpool,
            tc.tile_pool(name="vpool", bufs=4) as vpool,
            tc.tile_pool(name="tmp", bufs=2) as tpool,
            tc.tile_pool(name="minv", bufs=2) as mpool,
            tc.tile_pool(name="oout", bufs=2) as opool,
            tc.tile_pool(name="cpsum", bufs=8, space="PSUM") as cpsum,
        ):
            # warm the scalar-engine activation table before the params land
            warm = cpool.tile([128, 1], f32)
            nc.vector.memset(warm[:], 0.0)
            nc.scalar.activation(warm[:], warm[:], Prelu, bias=warm[:], scale=1.0, alpha=0.01)

            # ---- DMAs: MLP params on the HWDGE path; x grids and U0 via the
            # gpsimd SWDGE queue so they don't serialize behind the params ----
            pp1_sb = cpool.tile([128, _P1TOT], bf16)
            nc.sync.dma_start(pp1_sb[:], pp1_in[:])
            bias_sb = cpool.tile([128, _NBIAS], f32)
            nc.sync.dma_start(bias_sb[:], bias_in[:])
            pp2_sb = cpool.tile([128, _P2TOT], bf16)
            nc.sync.dma_start(pp2_sb[:], pp2_in[:])
            pp3_sb = cpool.tile([128, _P3TOT], bf16)
            nc.sync.dma_start(pp3_sb[:], pp3_in[:])

            grids = {}
            u0_sbs = {}

            def load_grid(b, ci_t):
                t = gpool.tile([128, GH * GW], f16, tag="g")
                nc.gpsimd.dma_start(t[:], xg_in[b, ci_t])
                grids[(b, ci_t)] = t[:].rearrange("p (h w) -> p h w", w=GW)

            load_grid(0, 0)
            for ci_t in range(CIT):
                t = cpool.tile([128, COT * UBF], f16, tag=f"u0{ci_t}")
                nc.gpsimd.dma_start(t[:], u0_in[ci_t])
                u0_sbs[ci_t] = t
            load_grid(0, 1)

            # ---- style MLP (identical to the direct-conv kernel) ----
            def mlp_layer(rhs_of_kt, kts, w_sb, w_base, bias_ap, out_sb):
                for ct in range(CIT):
                    mps = cpsum.tile([128, 512], f32, tag="ps")
                    for kt in range(kts):
                        nc.tensor.matmul(
                            mps[:, :BL],
                            w_sb[:, w_base + kt * C_IN + ct * 128 :][:, :128],
                            rhs_of_kt(kt),
                            start=(kt == 0),
                            stop=(kt == kts - 1),
                        )
                    nc.scalar.activation(
                        out_sb[:, ct * BL : (ct + 1) * BL],
                        mps[:, :BL],
                        Prelu,
                        bias=bias_ap(ct),
                        scale=1.0,
                        alpha=0.01,
                    )

            s0_sb = cpool.tile([128, CIT * BL], bf16)
            s1_sb = cpool.tile([128, CIT * BL], bf16)
            s_sb = cpool.tile([128, CIT * BL], f32)
            mlp_layer(
                lambda kt: pp1_sb[:, _PY + kt * BL : _PY + (kt + 1) * BL],
                4, pp1_sb, _PW0,
                lambda ct: bias_sb[:, ct : ct + 1],
                s0_sb,
            )
            mlp_layer(
                lambda kt: s0_sb[:, kt * BL : (kt + 1) * BL],
                2, pp2_sb, 0,
                lambda ct: bias_sb[:, CIT + ct : CIT + ct + 1],
                s1_sb,
            )
            mlp_layer(
                lambda kt: s1_sb[:, kt * BL : (kt + 1) * BL],
                2, pp3_sb, 0,
                lambda ct: bias_sb[:, 2 * CIT + ct : 2 * CIT + ct + 1],
                s_sb,
            )

            # ---- modulated transformed weights: Ub = U0 * s[b, ci] (fp16) ----
            ub = {}
            for b in range(BL):
                for ci_t in range(CIT):
                    t = cpool.tile([128, COT * UBF], f16, tag=f"ub{b}{ci_t}")
                    nc.vector.tensor_scalar_mul(
                        t[:], u0_sbs[ci_t][:],
                        s_sb[:, ci_t * BL + b : ci_t * BL + b + 1],
                    )
                    ub[(b, ci_t)] = t

            # ---- V = B^T d row-transform per (sample, ci-tile) ----
            vs = {}

            def build_v(b, ci_t):
                g = grids[(b, ci_t)]

                def d(p):          # rows p, p+4, ..., p+60  -> [128, 16, 66]
                    return g[:, p : p + 61 : 4, :]

                v = vpool.tile([128, UD * IT * GW], f16, tag="v")
                vv = v[:].rearrange("p (u i w) -> p u i w", i=IT, w=GW)

                def tmp(tag):
                    t = tpool.tile([128, IT * GW], f16, tag=tag)
                    return t[:].rearrange("p (i w) -> p i w", w=GW)

                t1, t2, t3, t4 = tmp("t1"), tmp("t2"), tmp("t3"), tmp("t4")
                t6, t7, t8 = tmp("t6"), tmp("t7"), tmp("t8")
                q1, q2, q3 = tmp("q1"), tmp("q2"), tmp("q3")
                tt = nc.vector.tensor_tensor
                tt(t1, d(1), d(2), ADD)
                tt(t3, d(1), d(2), SUB)
                nc.scalar.mul(q1, t1, 4.0)
                tt(t2, d(3), d(4), ADD)
                tt(vv[:, 1], q1, t2, SUB)          # V1 = 4(d1+d2)-(d3+d4) = -w1
                nc.scalar.mul(q2, t3, 4.0)
                tt(t4, d(3), d(4), SUB)
                tt(vv[:, 2], q2, t4, SUB)          # V2 = 4(d1-d2)-(d3-d4)
                tt(t6, d(3), d(1), SUB)
                nc.scalar.mul(q3, t6, 2.0)
                tt(t7, d(4), d(2), SUB)
                tt(vv[:, 3], q3, t7, ADD)          # V3 = 2(d3-d1)+(d4-d2)
                tt(vv[:, 4], t7, q3, SUB)          # V4 = (d4-d2)-2(d3-d1)
                nc.scalar.mul(q1, d(0), 4.0)
                nc.scalar.mul(q2, d(2), 5.0)
                tt(t8, q1, q2, SUB)
                tt(vv[:, 0], t8, d(4), ADD)        # V0 = 4d0-5d2+d4
                nc.scalar.mul(q1, d(1), 4.0)
                nc.scalar.mul(q2, d(3), 5.0)
                tt(t8, q1, q2, SUB)
                tt(vv[:, 5], t8, d(5), ADD)        # V5 = 4d1-5d3+d5
                vs[(b, ci_t)] = vv

            build_v(0, 0)
            build_v(0, 1)

            # ---- conv + inverse transform per (sample, co_t, chunk) ----
            def conv_chunk(b, co_t, ch, ci_major=False):
                ps = [
                    cpsum.tile([128, CHI * W], f32, name=f"ps_{b}_{co_t}_{ch}_{u}", tag="ps")
                    for u in range(UD)
                ]

                def mm(u, ci_t, kw):
                    nc.tensor.matmul(
                        ps[u][:],
                        ub[(b, ci_t)][:, ((co_t * UD + u) * K + kw) * 128 :][:, :128],
                        vs[(b, ci_t)][:, u, CHI * ch : CHI * ch + CHI, kw : kw + W],
                        start=(ci_t == 0 and kw == 0),
                        stop=(ci_t == CIT - 1 and kw == K - 1),
                    )

                if ci_major:
                    # first chunk: don't gate the whole chunk on the second V
                    for ci_t in range(CIT):
                        for kw in range(K):
                            for u in (1, 2, 3, 4, 0, 5):
                                mm(u, ci_t, kw)
                else:
                    # u-major: each P[u] completes after 6 matmuls so drains
                    # and the inverse pipeline alongside the later matmuls
                    for u in (1, 2, 3, 4, 0, 5):
                        for ci_t in range(CIT):
                            for kw in range(K):
                                mm(u, ci_t, kw)

                def mtmp(tag):
                    t = mpool.tile(
                        [128, CHI * W], f16, name=f"{tag}_{b}_{co_t}_{ch}", tag=tag
                    )
                    return t[:]

                # U row 1 and device V1 are both sign-flipped, so the PSUM
                # values are the true Winograd products and A^T applies
                # directly. m1/m2 drain on Act; r/s come straight out of PSUM
                # on gpsimd (saving the m3/m4 drains); m0/m5 drain on gpsimd.
                m1, m2, m0, m5 = mtmp("m1"), mtmp("m2"), mtmp("m0"), mtmp("m5")
                p_, q_, r_, s_ = mtmp("ip"), mtmp("iq"), mtmp("ir"), mtmp("is")
                s2, r4, s8 = mtmp("s2"), mtmp("r4"), mtmp("s8")
                oa, ob = mtmp("oa"), mtmp("ob")
                tt = nc.vector.tensor_tensor
                gt = nc.gpsimd.tensor_tensor
                nc.scalar.copy(m1, ps[1][:])
                nc.scalar.copy(m2, ps[2][:])
                gt(r_, ps[3][:], ps[4][:], ADD)
                gt(s_, ps[3][:], ps[4][:], SUB)
                nc.gpsimd.tensor_copy(m0, ps[0][:])
                nc.gpsimd.tensor_copy(m5, ps[5][:])
                tt(p_, m1, m2, ADD)                # p = m1 + m2
                tt(q_, m1, m2, SUB)                # q = m1 - m2
                nc.scalar.mul(s2, s_, 2.0)
                nc.scalar.mul(r4, r_, 4.0)
                nc.scalar.mul(s8, s_, 8.0)
                o = opool.tile([128, 4 * CHI * W], f16, tag="o")
                ov = o[:].rearrange("p (h w) -> p h w", w=W)
                nr = 4 * CHI                        # 32 output rows per chunk
                tt(oa, m0, p_, ADD)
                tt(ov[:, 0 : nr - 3 : 4, :], oa, r_, ADD)       # o0 = m0+p+r
                tt(ov[:, 1 : nr - 2 : 4, :], q_, s2, ADD)       # o1 = q+2s
                tt(ov[:, 2 : nr - 1 : 4, :], p_, r4, ADD)       # o2 = p+4r
                tt(ob, q_, m5, ADD)
                tt(ov[:, 3 : nr : 4, :], ob, s8, ADD)           # o3 = q+8s+m5
                nc.sync.dma_start(
                    out_ext[b, co_t][:, 2048 * ch : 2048 * (ch + 1)], o[:]
                )

            first = True
            for co_t in range(COT):
                if co_t == 1:
                    load_grid(1, 0)
                    build_v(1, 0)
                    load_grid(1, 1)
                    build_v(1, 1)
                for ch in range(NCH):
                    conv_chunk(0, co_t, ch, ci_major=first)
                    first = False
            for co_t in range(COT):
                for ch in range(NCH):
                    conv_chunk(1, co_t, ch)

    nc.compile()
    return nc


def _get_nc():
    global _COMPILED
    if _COMPILED is None:
        _COMPILED = _build()
    return _COMPILED


def _prep_in_maps(x, y, w0, b0, w1, b1, w2, b2, conv_w):
    import ml_dtypes

    BF = ml_dtypes.bfloat16
    x = np.ascontiguousarray(x, dtype=np.float32)
    y = np.ascontiguousarray(y, dtype=np.float32)

    # packed per-core-invariant MLP params (bf16 weights, fp32 biases)
    pp1_shared = np.empty((128, _P1TOT), dtype=BF)
    pp1_shared[:, _PW0 : _PW0 + 4 * C_IN] = (
        w0.astype(np.float32).T.reshape(4, 128, C_IN).transpose(1, 0, 2).reshape(128, 4 * C_IN)
    ).astype(BF)
    pp2 = np.ascontiguousarray(
        w1.astype(np.float32).T.reshape(2, 128, C_IN).transpose(1, 0, 2).reshape(128, 2 * C_IN)
    ).astype(BF)
    pp3 = np.ascontiguousarray(
        w2.astype(np.float32).T.reshape(2, 128, C_IN).transpose(1, 0, 2).reshape(128, 2 * C_IN)
    ).astype(BF)
    bias = np.empty((128, _NBIAS), dtype=np.float32)
    for i, bb in enumerate((b0, b1, b2)):
        bias[:, i * CIT : (i + 1) * CIT] = bb.astype(np.float32).reshape(CIT, 128).T

    # U0 = G @ conv_w along kh: (O,I,kh,kw) -> (I_t, ci, co_t, u, kw, co)
    T = np.einsum("uh,oihw->oiuw", _G, conv_w.astype(np.float64))
    u0 = np.ascontiguousarray(
        T.transpose(1, 2, 3, 0)
        .reshape(CIT, 128, UD, K, COT, 128)
        .transpose(0, 1, 4, 2, 3, 5)
        .reshape(CIT, 128, COT * UBF)
    ).astype(np.float16)

    xg_all = np.zeros((B, CIT, 128, GH, GW), dtype=np.float16)
    xg_all[:, :, :, 1 : H + 1, 1 : W + 1] = x.reshape(B, CIT, 128, H, W).astype(np.float16)
    xg_all = xg_all.reshape(B, CIT, 128, GH * GW)

    in_maps = []
    for c in range(NCORES):
        sl = slice(c * BL, (c + 1) * BL)
        pp1 = pp1_shared.copy()
        pp1[:, _PY : _PY + 4 * BL] = (
            y[sl].T.reshape(4, 128, BL).transpose(1, 0, 2).reshape(128, 4 * BL)
        ).astype(BF)
        in_maps.append(
            {
                "pp1": pp1,
                "pp2": pp2,
                "pp3": pp3,
                "bias": bias,
                "u0": u0,
                "xg": np.ascontiguousarray(xg_all[sl]),
            }
        )
    return in_maps


def _run(in_maps, trace=False):
    from concourse.bass_utils import run_bass_kernel_spmd

    nc = _get_nc()
    res = run_bass_kernel_spmd(nc, in_maps, list(range(NCORES)), trace=trace)
    out = np.concatenate(
        [
            res.results[c]["out"].astype(np.float32).reshape(BL, C_OUT, H, W)
            for c in range(NCORES)
        ],
        axis=0,
    )
    return out, res


def kernel(x, y, w0, b0, w1, b1, w2, b2, conv_w):
    in_maps = _prep_in_maps(x, y, w0, b0, w1, b1, w2, b2, conv_w)
    out, _ = _run(in_maps, trace=False)
    return out
